# revision 30
# baseline (speedup 1.0000x reference)
"""MiniMax-M2 MoE kernel for 8 Trainium2 NeuronCores.

Strategy (expert-parallel, sparse/routed, single device phase):
  Host: router gate matmul + sigmoid + top-4 + combine weights (fp32
    numpy); slot planner packs the 16 experts into S slots x 8 cores
    (experts may split across bins) minimizing the per-core token
    capacity CT; inputs are pre-quantized into fp8 hi/lo planes.
  Device (one SPMD kernel): per core, S expert slots of SwiGLU FFN over
    gathered tokens. All matmuls run as fp8 DoubleRow (2 K-chunks per
    instruction) using a 3-term hi/lo decomposition
      W.T X ~= Wh.T Xh + Wh.T Xl + Wl.T Xh
    with hi planes in e4m3 (scaled) and lo planes (residuals) in e5m2,
    which keeps accuracy at bf16 level while cutting PE time 25%.
  Host: scatter-add per-bin outputs into [T, H] with combine weight and
    fp8-scale descale folded into the multiply.
"""

import numpy as np
import ml_dtypes

import concourse.bass as bass  # noqa: F401  (kept for parity with env)
import concourse.tile as tile
from concourse import bacc, mybir
from concourse.bass_utils import run_bass_kernel_spmd

T, H, F, E, TOPK = 4096, 1024, 512, 16, 4
NCORES = 8
F32 = mybir.dt.float32
BF16 = mybir.dt.bfloat16
FP8H = mybir.dt.float8e4
FP8L = mybir.dt.float8e5
E4 = ml_dtypes.float8_e4m3
E5 = ml_dtypes.float8_e5m2
DR = mybir.MatmulPerfMode.DoubleRow

# power-of-2 scales keeping hi planes in e4m3 range (max 240)
SW1, SW3, SW2 = 128.0, 16.0, 128.0
DESCALE = 1.0 / (SW3 * SW2)

KP = H // 256   # stage-1 k-pairs per contraction (DoubleRow: 256 K/inst)
FC = F // 128   # stage-1 f-chunks (4)
FP2 = F // 256  # stage-2 f-pairs (2)
HC = H // 128   # output h-chunks (8)

_nc_cache: dict = {}
LAST_CAPS = (1024, 896, 320)  # caps used by the most recent kernel() call


def _pad64(n: int) -> int:
    return max(64, (n + 63) // 64 * 64)


def _chunks(cap: int) -> list[int]:
    n = -(-cap // 512)
    base, extra = divmod(cap, n)
    return [base + 1] * extra + [base] * (n - extra)


# ---------------------------------------------------------------------------
# Slot planner: pack experts (divisible) into S slots x 8 cores, one expert
# per bin, minimizing CT = sum(caps).
# ---------------------------------------------------------------------------

def _fit(counts, caps):
    """Try to pack experts into 8 bins per slot-cap. Returns assign
    (assign[core][slot] = (expert, tok_off, n) | None) or None."""
    import itertools
    from functools import lru_cache

    S = len(caps)
    items = sorted(((int(c), e) for e, c in enumerate(counts) if c > 0),
                   reverse=True)
    if not items:
        return tuple(caps), [[None] * S for _ in range(NCORES)]

    def patterns(k):
        out = []
        for m in itertools.product(range(5), repeat=S):
            tot = sum(m)
            if tot == 0 or tot > 4:
                continue
            cover = sum(mi * ci for mi, ci in zip(m, caps))
            if cover < k:
                continue
            if any(m[i] > 0 and cover - caps[i] >= k for i in range(S)):
                continue  # not minimal
            out.append(m)
        return out

    pats = [patterns(k) for k, _ in items]
    if any(not p for p in pats):
        return None

    @lru_cache(maxsize=None)
    def solve(i, used):
        if i == len(items):
            return ()
        for m in pats[i]:
            nu = tuple(u + mi for u, mi in zip(used, m))
            if any(u > NCORES for u in nu):
                continue
            r = solve(i + 1, nu)
            if r is not None:
                return (m,) + r
        return None

    sol = solve(0, (0,) * S)
    if sol is None:
        return None

    # refine caps to exact integers for this split structure:
    # for each expert, sum_s m_s * cap_s >= count must hold
    caps = list(caps)
    for _ in range(12):
        changed = False
        for s in range(S):
            need = 0
            for (k, e), m in zip(items, sol):
                if m[s] == 0:
                    continue
                rest = sum(mi * ci for i, (mi, ci) in
                           enumerate(zip(m, caps)) if i != s)
                need = max(need, -(-(k - rest) // m[s]))
            need = max(need, 1)
            if need < caps[s]:
                caps[s] = need
                changed = True
        if not changed:
            break
    caps = tuple(caps)

    bins: dict[int, list] = {s: [] for s in range(S)}
    slot_order = sorted(range(S), key=lambda s: -caps[s])
    for (k, e), m in zip(items, sol):
        rem, toff = k, 0
        for s in slot_order:
            for _ in range(m[s]):
                take = min(caps[s], rem)
                if take > 0:
                    bins[s].append((e, toff, take))
                    toff += take
                    rem -= take
        if rem > 0:
            return None
    if any(len(b) > NCORES for b in bins.values()):
        return None
    assign = [[None] * S for _ in range(NCORES)]
    for s in range(S):
        for c, bn in enumerate(bins[s]):
            assign[c][s] = bn
    return caps, assign


def _plan_slots(counts):
    """Choose slot caps + bin assignment. Returns (caps, assign)."""
    counts = np.asarray(counts, dtype=np.int64)
    order = np.argsort(-counts, kind="stable")

    # classic 2-slot fallback (always feasible)
    c0 = _pad64(int(counts[order[0]]))
    c1 = _pad64(int(counts[order[NCORES]])) if E > NCORES else 64
    caps_fb = (c0, c1)
    assign_fb = [[None] * 2 for _ in range(NCORES)]
    for c in range(NCORES):
        for s in range(2):
            e = int(order[s * NCORES + c])
            if counts[e] > 0:
                assign_fb[c][s] = (e, 0, int(counts[e]))
    SLOT_PENALTY = 48  # tokens-equivalent per extra slot
    best = (caps_fb, assign_fb)
    best_cost = (sum(caps_fb) + 2 * SLOT_PENALTY
                 + 24 * sum(-(-c // 512) for c in caps_fb))

    total = int(counts.sum())
    ct_lo = _pad64((total + NCORES - 1) // NCORES)
    for ct in range(ct_lo, sum(caps_fb) + 64, 64):
        if ct - 128 + 2 * SLOT_PENALTY >= best_cost:
            break
        for S in (2, 3):
            # partitions of ct into S descending multiples of 64
            if S == 2:
                parts = [(a, ct - a) for a in range(_pad64(ct // 2), ct, 64)
                         if ct - a >= 64]
            else:
                parts = []
                for a in range(_pad64(ct // 3), ct - 127, 64):
                    for b in range(_pad64((ct - a + 1) // 2),
                                   min(a, ct - a - 63) + 1, 64):
                        cc = ct - a - b
                        if 64 <= cc <= b:
                            parts.append((a, b, cc))
            for caps in parts:
                r = _fit(counts, caps)
                if r is None:
                    continue
                rcaps, assign = r
                cost = (sum(rcaps) + S * SLOT_PENALTY
                        + 24 * sum(-(-c // 512) for c in rcaps))
                if cost < best_cost:
                    best, best_cost = (rcaps, assign), cost

    # local sweeps around the best caps: structures feasible only at off-64
    # capacities (splits land tighter) show up here
    def sweep(b, radius, step):
        nonlocal best, best_cost
        b0, b1, b2 = b
        for c0 in range(max(256, b0 - radius), b0 + radius + 1, step):
            for c1 in range(max(96, b1 - radius),
                            min(c0, b1 + radius) + 1, step):
                for c2 in range(max(32, b2 - radius - 64),
                                min(c1, b2 + radius) + 1, step):
                    r = _fit(counts, (c0, c1, c2))
                    if r is None:
                        continue
                    rcaps, assign = r
                    cost = (sum(rcaps) + 3 * SLOT_PENALTY
                            + 24 * sum(-(-c // 512) for c in rcaps))
                    if cost < best_cost:
                        best, best_cost = (rcaps, assign), cost

    # seed candidates (best known for the reference workload's counts)
    for seed in ((1033, 924, 181), (1033, 887, 256), (1004, 866, 317)):
        r = _fit(counts, seed)
        if r is not None:
            rcaps, assign = r
            cost = (sum(rcaps) + 3 * SLOT_PENALTY
                    + 24 * sum(-(-c // 512) for c in rcaps))
            if cost < best_cost:
                best, best_cost = (rcaps, assign), cost

    if len(best[0]) == 3:
        sweep(best[0], 160, 32)
        if len(best[0]) == 3:
            sweep(best[0], 32, 8)
    return best


# ---------------------------------------------------------------------------
# Device kernel: S-slot SwiGLU FFN, fp8 DoubleRow 3-term
# ---------------------------------------------------------------------------

def _build_ffn(caps):
    S = len(caps)
    CT = sum(caps)
    nc = bacc.Bacc("TRN2", target_bir_lowering=False, debug=False,
                   num_devices=NCORES)
    w13h_d, w13l_d, w2h_d, w2l_d = [], [], [], []
    for s in range(S):
        w13h_d.append(nc.dram_tensor(f"w13h{s}", [H, 2 * F], FP8H,
                                     kind="ExternalInput").ap())
        w13l_d.append(nc.dram_tensor(f"w13l{s}", [H, 2 * F], FP8L,
                                     kind="ExternalInput").ap())
        w2h_d.append(nc.dram_tensor(f"w2h{s}", [F, H], FP8H,
                                    kind="ExternalInput").ap())
        w2l_d.append(nc.dram_tensor(f"w2l{s}", [F, H], FP8L,
                                    kind="ExternalInput").ap())
    CTP = CT + 512  # x planes padded so chunk loads are always 512 wide
    xqh = nc.dram_tensor("xqh", [H, CTP], FP8H, kind="ExternalInput").ap()
    xql = nc.dram_tensor("xql", [H, CTP], FP8L, kind="ExternalInput").ap()
    ygt = nc.dram_tensor("ygt", [H, CT], BF16, kind="ExternalOutput").ap()

    xqh_r = xqh.rearrange("(kp two p) t -> p kp two t", p=128, two=2)
    xql_r = xql.rearrange("(kp two p) t -> p kp two t", p=128, two=2)
    ygt_r = ygt.rearrange("(hc p) t -> p hc t", p=128)
    SILU = mybir.ActivationFunctionType.Silu
    COPY = mybir.ActivationFunctionType.Copy

    with tile.TileContext(nc) as tc:
        with (
            tc.tile_pool(name="w13_p", bufs=2) as w13_p,
            tc.tile_pool(name="w2_p", bufs=2) as w2_p,
            tc.tile_pool(name="xg_p", bufs=3) as xg_p,
            tc.tile_pool(name="sg_p", bufs=3) as sg_p,
            tc.tile_pool(name="h32_p", bufs=3) as h32_p,
            tc.tile_pool(name="h8_p", bufs=3) as h8_p,
            tc.tile_pool(name="y_p", bufs=2) as y_p,
            tc.tile_pool(name="ps", bufs=8, space="PSUM") as ps_pool,
        ):
            # flat chunk list: (slot, chunk-in-slot, t0, tl)
            chunk_list = []
            for s in range(S):
                cap = caps[s]
                chs = _chunks(cap)
                if s == 0 and cap > 384:
                    # small leading chunk warms the pipeline sooner
                    chs = [256] + _chunks(cap - 256)
                if s == S - 1 and cap > 384:
                    # small final chunk shrinks the end-of-kernel drain tail
                    chs = _chunks(cap - 128) + [128]
                t0 = sum(caps[:s])
                for ci, tl in enumerate(chs):
                    chunk_list.append((s, ci, t0, tl))
                    t0 += tl

            slot_w = {}

            def load_slot_weights(s):
                wh13 = w13_p.tile([128, KP, 2, 2 * F], FP8H, tag="w13h",
                                  name=f"wh13_{s}")
                wl13 = w13_p.tile([128, KP, 2, 2 * F], FP8L, tag="w13l",
                                  name=f"wl13_{s}")
                wh2 = w2_p.tile([128, FP2, 2, H], FP8H, tag="w2h",
                                name=f"wh2_{s}")
                wl2 = w2_p.tile([128, FP2, 2, H], FP8L, tag="w2l",
                                name=f"wl2_{s}")
                w13h_r = w13h_d[s].rearrange("(kp two p) f -> p kp two f",
                                             p=128, two=2)
                w13l_r = w13l_d[s].rearrange("(kp two p) f -> p kp two f",
                                             p=128, two=2)
                if s == 0:
                    # fine-grained startup loads so kp0 matmuls start early;
                    # half 0 (pass-A columns) streams first
                    for hf in range(2):
                        for kp in range(KP):
                            nc.gpsimd.dma_start(
                                wh13[:, kp, :, hf * 512:(hf + 1) * 512],
                                w13h_r[:, kp, :, hf * 512:(hf + 1) * 512])
                            nc.scalar.dma_start(
                                wl13[:, kp, :, hf * 512:(hf + 1) * 512],
                                w13l_r[:, kp, :, hf * 512:(hf + 1) * 512])
                else:
                    nc.gpsimd.dma_start(wh13[:], w13h_r)
                    nc.gpsimd.dma_start(wl13[:], w13l_r)
                nc.gpsimd.dma_start(
                    wh2[:], w2h_d[s].rearrange("(fp two p) h -> p fp two h",
                                               p=128, two=2))
                nc.gpsimd.dma_start(
                    wl2[:], w2l_d[s].rearrange("(fp two p) h -> p fp two h",
                                               p=128, two=2))
                slot_w[s] = (wh13, wl13, wh2, wl2)

            def stage1(s, ci, t0, tl):
                wh13, wl13, _, _ = slot_w[s]
                xh_sb = xg_p.tile([128, KP, 2, 512], FP8H, tag="xh",
                                  name=f"xh_sb_{s}_{ci}")
                xl_sb = xg_p.tile([128, KP, 2, 512], FP8L, tag="xl",
                                  name=f"xl_sb_{s}_{ci}")
                nc.sync.dma_start(xh_sb[:], xqh_r[:, :, :, t0:t0 + 512])
                nc.sync.dma_start(xl_sb[:], xql_r[:, :, :, t0:t0 + 512])

                hh_sb = h8_p.tile([128, FC, 512], FP8H, tag="hh",
                                  name=f"hh_sb_{s}_{ci}")
                hl_sb = h8_p.tile([128, FC, 512], FP8L, tag="hl",
                                  name=f"hl_sb_{s}_{ci}")

                terms = ((wh13, xh_sb), (wh13, xl_sb), (wl13, xh_sb))
                for pa in range(2):  # two fi passes -> 4 PSUM banks each
                    fis = (0, 1) if pa == 0 else (2, 3)
                    psg = {fi: ps_pool.tile([128, 512], F32, tag="ps",
                                            name=f"psg_{s}_{ci}_{fi}")
                           for fi in fis}
                    psu = {fi: ps_pool.tile([128, 512], F32, tag="ps",
                                            name=f"psu_{s}_{ci}_{fi}")
                           for fi in fis}

                    def s1(fi, kp, ti, wsb, xsb):
                        first = (kp == 0 and ti == 0)
                        last = (kp == KP - 1 and ti == 2)
                        gc = (fi // 2) * 512 + (fi % 2) * 128
                        uc = gc + 256
                        nc.tensor.matmul(
                            psg[fi][:, :tl],
                            lhsT=wsb[:, kp, :, gc:gc + 128],
                            rhs=xsb[:, kp, :, :tl],
                            start=first, stop=last, perf_mode=DR)
                        nc.tensor.matmul(
                            psu[fi][:, :tl],
                            lhsT=wsb[:, kp, :, uc:uc + 128],
                            rhs=xsb[:, kp, :, :tl],
                            start=first, stop=last, perf_mode=DR)

                    if s == 0 and ci == 0:
                        # term-major: WhXh first (loads land earliest),
                        # then WlXh, then WhXl
                        t0_terms = ((wh13, xh_sb), (wl13, xh_sb),
                                    (wh13, xl_sb))
                        for ti, (wsb, xsb) in enumerate(t0_terms):
                            for kp in range(KP):
                                for fi in fis:
                                    s1(fi, kp, ti, wsb, xsb)
                    else:
                        for fi in fis:
                            for kp in range(KP):
                                for ti, (wsb, xsb) in enumerate(terms):
                                    s1(fi, kp, ti, wsb, xsb)

                    for fi in fis:
                        sg = sg_p.tile([128, 512], F32, tag="sg",
                                       name=f"sg_{s}_{ci}_{fi}")
                        nc.scalar.activation(sg[:, :tl], psg[fi][:, :tl],
                                             SILU, scale=1.0 / SW1)
                        h32 = h32_p.tile([128, 512], F32, tag="h32",
                                         name=f"h32_{s}_{ci}_{fi}")
                        nc.vector.tensor_tensor(
                            h32[:, :tl], sg[:, :tl], psu[fi][:, :tl],
                            mybir.AluOpType.mult)
                        # identity on-device (|h|<=110); keeps the fp8
                        # cast finite under cost-model simulation where
                        # DoubleRow values are approximate
                        nc.vector.tensor_scalar(
                            h32[:, :tl], h32[:, :tl], 224.0, -224.0,
                            op0=mybir.AluOpType.min,
                            op1=mybir.AluOpType.max)
                        nc.vector.tensor_copy(hh_sb[:, fi, :tl],
                                               h32[:, :tl])
                        nc.vector.tensor_tensor(
                            hl_sb[:, fi, :tl], h32[:, :tl],
                            hh_sb[:, fi, :tl], mybir.AluOpType.subtract)
                return hh_sb, hl_sb

            def stage2(s, ci, t0, tl, hh_sb, hl_sb):
                # transposed: yT[h, t] = sum_f w2T[f, h] h[f, t]
                _, _, wh2, wl2 = slot_w[s]
                y_lo = y_p.tile([128, HC // 2, 512], BF16, tag="ylo",
                                name=f"y_lo_{s}_{ci}")
                y_hi = y_p.tile([128, HC // 2, 512], BF16, tag="yhi",
                                name=f"y_hi_{s}_{ci}")
                t2 = ((wh2, hh_sb), (wh2, hl_sb), (wl2, hh_sb))
                for hc in range(HC):
                    psy = ps_pool.tile([128, 512], F32, tag="ps",
                                       name=f"psy_{s}_{ci}_{hc}")
                    i = 0
                    for j in range(FP2):
                        for (w2sb, hsb) in t2:
                            nc.tensor.matmul(
                                psy[:, :tl],
                                lhsT=w2sb[:, j, :,
                                          hc * 128:(hc + 1) * 128],
                                rhs=hsb[:, 2 * j:2 * j + 2, :tl],
                                start=(i == 0), stop=(i == 3 * FP2 - 1),
                                perf_mode=DR)
                            i += 1
                    y_sb = y_lo if hc < HC // 2 else y_hi
                    yc = hc % (HC // 2)
                    if hc % 2 == 0:
                        nc.scalar.activation(y_sb[:, yc, :tl],
                                             psy[:, :tl], COPY)
                    else:
                        nc.vector.tensor_copy(y_sb[:, yc, :tl],
                                              psy[:, :tl])
                nc.gpsimd.dma_start(ygt_r[:, :HC // 2, t0:t0 + tl],
                                    y_lo[:, :, :tl])
                nc.sync.dma_start(ygt_r[:, HC // 2:, t0:t0 + tl],
                                  y_hi[:, :, :tl])

            # software pipeline: stage2 lags one chunk behind stage1 so the
            # PE always has stage-1 work while h planes drain
            load_slot_weights(0)
            prev = None
            for (s, ci, t0, tl) in chunk_list:
                h_pair = stage1(s, ci, t0, tl)
                if ci == 0 and s + 1 < S:
                    load_slot_weights(s + 1)
                if prev is not None:
                    stage2(*prev)
                prev = (s, ci, t0, tl) + h_pair
            stage2(*prev)

    nc.compile()
    return nc


def _ffn_nc(caps):
    key = ("ffn", caps)
    if key not in _nc_cache:
        _nc_cache[key] = _build_ffn(caps)
    return _nc_cache[key]


# ---------------------------------------------------------------------------
# Host orchestration
# ---------------------------------------------------------------------------

def kernel(hidden_states, gate_w, bias, w1, w3, w2):
    x = np.ascontiguousarray(np.asarray(hidden_states, dtype=np.float32))
    gate_w = np.asarray(gate_w, dtype=np.float32)
    bias = np.asarray(bias, dtype=np.float32)
    w1 = np.asarray(w1, dtype=np.float32)
    w3 = np.asarray(w3, dtype=np.float32)
    w2 = np.asarray(w2, dtype=np.float32)

    # ---- Routing on host (fp32) ----
    logits = x @ gate_w.T                               # [T, E]
    scores = 1.0 / (1.0 + np.exp(-logits))
    biased = scores + bias[None, :]
    topi = np.argpartition(-biased, TOPK - 1, axis=1)[:, :TOPK]  # [T, K]
    topw = np.take_along_axis(scores, topi, axis=1)
    topw = topw / topw.sum(axis=1, keepdims=True)
    combine = np.zeros((T, E), dtype=np.float32)
    np.put_along_axis(combine, topi, topw, axis=1)
    idx_per_e = [np.nonzero(combine[:, e] > 0.0)[0] for e in range(E)]
    counts = np.array([len(ix) for ix in idx_per_e])

    # ---- Slot planning ----
    caps, assign = _plan_slots(counts)
    global LAST_CAPS
    LAST_CAPS = caps
    S = len(caps)
    CT = sum(caps)

    # ---- hi/lo fp8 planes ----
    xT = np.ascontiguousarray(x.T)                      # [H, T]
    xTh = xT.astype(E4)
    xTl = (xT - xTh.astype(np.float32)).astype(E5)

    experts_used = sorted({bn[0] for core in assign for bn in core if bn})
    wplanes = {}
    for e in experts_used:
        # half-major layout: [g(0:256) u(0:256) g(256:512) u(256:512)]
        w1t = w1[e].T * SW1
        w3t = w3[e].T * SW3
        w13 = np.concatenate([w1t[:, :256], w3t[:, :256],
                              w1t[:, 256:], w3t[:, 256:]],
                             axis=1)                    # [H, 2F]
        w13h = w13.astype(E4)
        w13l = (w13 - w13h.astype(np.float32)).astype(E5)
        w2t = np.ascontiguousarray(w2[e].T) * SW2       # [F, H]
        w2h = w2t.astype(E4)
        w2l = (w2t - w2h.astype(np.float32)).astype(E5)
        wplanes[e] = (np.ascontiguousarray(w13h), np.ascontiguousarray(w13l),
                      np.ascontiguousarray(w2h), np.ascontiguousarray(w2l))
    e0 = experts_used[0] if experts_used else 0
    if e0 not in wplanes:
        z13 = np.zeros((H, 2 * F), dtype=E4)
        wplanes[e0] = (z13, np.zeros((H, 2 * F), dtype=E5),
                       np.zeros((F, H), dtype=E4), np.zeros((F, H), dtype=E5))

    # ---- Per-core inputs ----
    in_maps = []
    for c in range(NCORES):
        idx_pad = np.zeros(CT, dtype=np.int64)
        for s in range(S):
            bn = assign[c][s]
            if bn is None:
                continue
            e, toff, n = bn
            soff = sum(caps[:s])
            idx_pad[soff:soff + n] = idx_per_e[e][toff:toff + n]
        idx_full = np.zeros(CT + 512, dtype=np.int64)
        idx_full[:CT] = idx_pad
        im = {
            "xqh": np.ascontiguousarray(xTh[:, idx_full]),
            "xql": np.ascontiguousarray(xTl[:, idx_full]),
        }
        for s in range(S):
            bn = assign[c][s]
            e = bn[0] if bn is not None else e0
            p13h, p13l, p2h, p2l = wplanes[e]
            im[f"w13h{s}"] = p13h
            im[f"w13l{s}"] = p13l
            im[f"w2h{s}"] = p2h
            im[f"w2l{s}"] = p2l
        in_maps.append(im)

    # ---- Device FFN ----
    ncf = _ffn_nc(caps)
    res = run_bass_kernel_spmd(ncf, in_maps, core_ids=list(range(NCORES)))

    # ---- Host combine + scatter-add ----
    out = np.zeros((T, H), dtype=np.float32)
    for c in range(NCORES):
        ygt = res.results[c]["ygt"]                     # [H, CT] bf16
        for s in range(S):
            bn = assign[c][s]
            if bn is None:
                continue
            e, toff, n = bn
            soff = sum(caps[:s])
            ix = idx_per_e[e][toff:toff + n]
            yrows = ygt[:, soff:soff + n].astype(np.float32).T  # [n, H]
            out[ix] += (combine[ix, e] * DESCALE)[:, None] * yrows
    return out


# revision 35
# speedup vs baseline: 1.0051x; 1.0051x over previous
"""MiniMax-M2 MoE kernel for 8 Trainium2 NeuronCores.

Strategy (expert-parallel, sparse/routed, single device phase):
  Host: router gate matmul + sigmoid + top-4 + combine weights (fp32
    numpy); slot planner packs the 16 experts into S slots x 8 cores
    (experts may split across bins) minimizing the per-core token
    capacity CT; inputs are pre-quantized into fp8 hi/lo planes.
  Device (one SPMD kernel): per core, S expert slots of SwiGLU FFN over
    gathered tokens. All matmuls run as fp8 DoubleRow (2 K-chunks per
    instruction) using a 3-term hi/lo decomposition
      W.T X ~= Wh.T Xh + Wh.T Xl + Wl.T Xh
    with hi planes in e4m3 (scaled) and lo planes (residuals) in e5m2,
    which keeps accuracy at bf16 level while cutting PE time 25%.
  Host: scatter-add per-bin outputs into [T, H] with combine weight and
    fp8-scale descale folded into the multiply.
"""

import numpy as np
import ml_dtypes

import concourse.bass as bass  # noqa: F401  (kept for parity with env)
import concourse.tile as tile
from concourse import bacc, mybir
from concourse.bass_utils import run_bass_kernel_spmd

T, H, F, E, TOPK = 4096, 1024, 512, 16, 4
NCORES = 8
F32 = mybir.dt.float32
BF16 = mybir.dt.bfloat16
FP8H = mybir.dt.float8e4
FP8L = mybir.dt.float8e5
E4 = ml_dtypes.float8_e4m3
E5 = ml_dtypes.float8_e5m2
DR = mybir.MatmulPerfMode.DoubleRow

# power-of-2 scales keeping hi planes in e4m3 range (max 240)
SW1, SW3, SW2 = 128.0, 16.0, 128.0
DESCALE = 1.0 / (SW3 * SW2)

KP = H // 256   # stage-1 k-pairs per contraction (DoubleRow: 256 K/inst)
FC = F // 128   # stage-1 f-chunks (4)
FP2 = F // 256  # stage-2 f-pairs (2)
HC = H // 128   # output h-chunks (8)

_nc_cache: dict = {}
LAST_CAPS = (1024, 896, 320)  # caps used by the most recent kernel() call


def _pad64(n: int) -> int:
    return max(64, (n + 63) // 64 * 64)


def _chunks(cap: int) -> list[int]:
    n = -(-cap // 512)
    base, extra = divmod(cap, n)
    return [base + 1] * extra + [base] * (n - extra)


# ---------------------------------------------------------------------------
# Slot planner: pack experts (divisible) into S slots x 8 cores, one expert
# per bin, minimizing CT = sum(caps).
# ---------------------------------------------------------------------------

def _fit(counts, caps):
    """Try to pack experts into 8 bins per slot-cap. Returns assign
    (assign[core][slot] = (expert, tok_off, n) | None) or None."""
    import itertools
    from functools import lru_cache

    S = len(caps)
    items = sorted(((int(c), e) for e, c in enumerate(counts) if c > 0),
                   reverse=True)
    if not items:
        return tuple(caps), [[None] * S for _ in range(NCORES)]

    def patterns(k):
        out = []
        for m in itertools.product(range(5), repeat=S):
            tot = sum(m)
            if tot == 0 or tot > 4:
                continue
            cover = sum(mi * ci for mi, ci in zip(m, caps))
            if cover < k:
                continue
            if any(m[i] > 0 and cover - caps[i] >= k for i in range(S)):
                continue  # not minimal
            out.append(m)
        return out

    pats = [patterns(k) for k, _ in items]
    if any(not p for p in pats):
        return None

    @lru_cache(maxsize=None)
    def solve(i, used):
        if i == len(items):
            return ()
        for m in pats[i]:
            nu = tuple(u + mi for u, mi in zip(used, m))
            if any(u > NCORES for u in nu):
                continue
            r = solve(i + 1, nu)
            if r is not None:
                return (m,) + r
        return None

    sol = solve(0, (0,) * S)
    if sol is None:
        return None

    # refine caps to exact integers for this split structure:
    # for each expert, sum_s m_s * cap_s >= count must hold
    caps = list(caps)
    for _ in range(12):
        changed = False
        for s in range(S):
            need = 0
            for (k, e), m in zip(items, sol):
                if m[s] == 0:
                    continue
                rest = sum(mi * ci for i, (mi, ci) in
                           enumerate(zip(m, caps)) if i != s)
                need = max(need, -(-(k - rest) // m[s]))
            need = max(need, 1)
            if need < caps[s]:
                caps[s] = need
                changed = True
        if not changed:
            break
    caps = tuple(caps)

    bins: dict[int, list] = {s: [] for s in range(S)}
    slot_order = sorted(range(S), key=lambda s: -caps[s])
    for (k, e), m in zip(items, sol):
        rem, toff = k, 0
        for s in slot_order:
            for _ in range(m[s]):
                take = min(caps[s], rem)
                if take > 0:
                    bins[s].append((e, toff, take))
                    toff += take
                    rem -= take
        if rem > 0:
            return None
    if any(len(b) > NCORES for b in bins.values()):
        return None
    assign = [[None] * S for _ in range(NCORES)]
    for s in range(S):
        for c, bn in enumerate(bins[s]):
            assign[c][s] = bn
    return caps, assign


def _plan_slots(counts):
    """Choose slot caps + bin assignment. Returns (caps, assign)."""
    counts = np.asarray(counts, dtype=np.int64)
    order = np.argsort(-counts, kind="stable")

    # classic 2-slot fallback (always feasible)
    c0 = _pad64(int(counts[order[0]]))
    c1 = _pad64(int(counts[order[NCORES]])) if E > NCORES else 64
    caps_fb = (c0, c1)
    assign_fb = [[None] * 2 for _ in range(NCORES)]
    for c in range(NCORES):
        for s in range(2):
            e = int(order[s * NCORES + c])
            if counts[e] > 0:
                assign_fb[c][s] = (e, 0, int(counts[e]))
    SLOT_PENALTY = 48  # tokens-equivalent per extra slot
    best = (caps_fb, assign_fb)
    best_cost = (sum(caps_fb) + 2 * SLOT_PENALTY
                 + 24 * sum(-(-c // 512) for c in caps_fb))

    total = int(counts.sum())
    ct_lo = _pad64((total + NCORES - 1) // NCORES)
    for ct in range(ct_lo, sum(caps_fb) + 64, 64):
        if ct - 128 + 2 * SLOT_PENALTY >= best_cost:
            break
        for S in (2, 3):
            # partitions of ct into S descending multiples of 64
            if S == 2:
                parts = [(a, ct - a) for a in range(_pad64(ct // 2), ct, 64)
                         if ct - a >= 64]
            else:
                parts = []
                for a in range(_pad64(ct // 3), ct - 127, 64):
                    for b in range(_pad64((ct - a + 1) // 2),
                                   min(a, ct - a - 63) + 1, 64):
                        cc = ct - a - b
                        if 64 <= cc <= b:
                            parts.append((a, b, cc))
            for caps in parts:
                r = _fit(counts, caps)
                if r is None:
                    continue
                rcaps, assign = r
                cost = (sum(rcaps) + S * SLOT_PENALTY
                        + 24 * sum(-(-c // 512) for c in rcaps))
                if cost < best_cost:
                    best, best_cost = (rcaps, assign), cost

    # local sweeps around the best caps: structures feasible only at off-64
    # capacities (splits land tighter) show up here
    def sweep(b, radius, step):
        nonlocal best, best_cost
        b0, b1, b2 = b
        for c0 in range(max(256, b0 - radius), b0 + radius + 1, step):
            for c1 in range(max(96, b1 - radius),
                            min(c0, b1 + radius) + 1, step):
                for c2 in range(max(32, b2 - radius - 64),
                                min(c1, b2 + radius) + 1, step):
                    r = _fit(counts, (c0, c1, c2))
                    if r is None:
                        continue
                    rcaps, assign = r
                    cost = (sum(rcaps) + 3 * SLOT_PENALTY
                            + 24 * sum(-(-c // 512) for c in rcaps))
                    if cost < best_cost:
                        best, best_cost = (rcaps, assign), cost

    # seed candidates (best known for the reference workload's counts)
    for seed in ((1033, 924, 181), (1033, 887, 256), (1004, 866, 317)):
        r = _fit(counts, seed)
        if r is not None:
            rcaps, assign = r
            cost = (sum(rcaps) + 3 * SLOT_PENALTY
                    + 24 * sum(-(-c // 512) for c in rcaps))
            if cost < best_cost:
                best, best_cost = (rcaps, assign), cost

    if len(best[0]) == 3:
        sweep(best[0], 160, 32)
        if len(best[0]) == 3:
            sweep(best[0], 32, 8)
    return best


# ---------------------------------------------------------------------------
# Device kernel: S-slot SwiGLU FFN, fp8 DoubleRow 3-term
# ---------------------------------------------------------------------------

def _build_ffn(caps):
    S = len(caps)
    CT = sum(caps)
    nc = bacc.Bacc("TRN2", target_bir_lowering=False, debug=False,
                   num_devices=NCORES)
    w13h_d, w13l_d, w2h_d, w2l_d = [], [], [], []
    for s in range(S):
        w13h_d.append(nc.dram_tensor(f"w13h{s}", [H, 2 * F], FP8H,
                                     kind="ExternalInput").ap())
        w13l_d.append(nc.dram_tensor(f"w13l{s}", [H, 2 * F], FP8L,
                                     kind="ExternalInput").ap())
        w2h_d.append(nc.dram_tensor(f"w2h{s}", [F, H], FP8H,
                                    kind="ExternalInput").ap())
        w2l_d.append(nc.dram_tensor(f"w2l{s}", [F, H], FP8L,
                                    kind="ExternalInput").ap())
    CTP = CT + 512  # x planes padded so chunk loads are always 512 wide
    xqh = nc.dram_tensor("xqh", [H, CTP], FP8H, kind="ExternalInput").ap()
    xql = nc.dram_tensor("xql", [H, CTP], FP8L, kind="ExternalInput").ap()
    ygt = nc.dram_tensor("ygt", [H, CT], BF16, kind="ExternalOutput").ap()

    xqh_r = xqh.rearrange("(kp two p) t -> p kp two t", p=128, two=2)
    xql_r = xql.rearrange("(kp two p) t -> p kp two t", p=128, two=2)
    ygt_r = ygt.rearrange("(hc p) t -> p hc t", p=128)
    SILU = mybir.ActivationFunctionType.Silu
    COPY = mybir.ActivationFunctionType.Copy

    with tile.TileContext(nc) as tc:
        with (
            tc.tile_pool(name="w13_p", bufs=2) as w13_p,
            tc.tile_pool(name="w2_p", bufs=2) as w2_p,
            tc.tile_pool(name="xg_p", bufs=3) as xg_p,
            tc.tile_pool(name="sg_p", bufs=3) as sg_p,
            tc.tile_pool(name="h32_p", bufs=3) as h32_p,
            tc.tile_pool(name="h8_p", bufs=3) as h8_p,
            tc.tile_pool(name="y_p", bufs=2) as y_p,
            tc.tile_pool(name="ps", bufs=8, space="PSUM") as ps_pool,
        ):
            # flat chunk list: (slot, chunk-in-slot, t0, tl)
            chunk_list = []
            for s in range(S):
                cap = caps[s]
                chs = _chunks(cap)
                if s == 0 and cap > 384:
                    # small leading chunk warms the pipeline sooner
                    chs = [256] + _chunks(cap - 256)
                if s == S - 1 and cap > 384:
                    # small final chunk shrinks the end-of-kernel drain tail
                    chs = _chunks(cap - 128) + [128]
                t0 = sum(caps[:s])
                for ci, tl in enumerate(chs):
                    chunk_list.append((s, ci, t0, tl))
                    t0 += tl

            slot_w = {}

            def load_slot_weights(s):
                wh13 = w13_p.tile([128, KP, 2, 2 * F], FP8H, tag="w13h",
                                  name=f"wh13_{s}")
                wl13 = w13_p.tile([128, KP, 2, 2 * F], FP8L, tag="w13l",
                                  name=f"wl13_{s}")
                wh2 = w2_p.tile([128, FP2, 2, H], FP8H, tag="w2h",
                                name=f"wh2_{s}")
                wl2 = w2_p.tile([128, FP2, 2, H], FP8L, tag="w2l",
                                name=f"wl2_{s}")
                w13h_r = w13h_d[s].rearrange("(kp two p) f -> p kp two f",
                                             p=128, two=2)
                w13l_r = w13l_d[s].rearrange("(kp two p) f -> p kp two f",
                                             p=128, two=2)
                if s == 0:
                    # fine-grained startup loads so kp0 matmuls start early;
                    # half 0 (pass-A columns) streams first
                    for hf in range(2):
                        for kp in range(KP):
                            nc.gpsimd.dma_start(
                                wh13[:, kp, :, hf * 512:(hf + 1) * 512],
                                w13h_r[:, kp, :, hf * 512:(hf + 1) * 512])
                            nc.scalar.dma_start(
                                wl13[:, kp, :, hf * 512:(hf + 1) * 512],
                                w13l_r[:, kp, :, hf * 512:(hf + 1) * 512])
                else:
                    nc.gpsimd.dma_start(wh13[:], w13h_r)
                    nc.gpsimd.dma_start(wl13[:], w13l_r)
                nc.gpsimd.dma_start(
                    wh2[:], w2h_d[s].rearrange("(fp two p) h -> p fp two h",
                                               p=128, two=2))
                nc.gpsimd.dma_start(
                    wl2[:], w2l_d[s].rearrange("(fp two p) h -> p fp two h",
                                               p=128, two=2))
                slot_w[s] = (wh13, wl13, wh2, wl2)

            def stage1(s, ci, t0, tl):
                wh13, wl13, _, _ = slot_w[s]
                xh_sb = xg_p.tile([128, KP, 2, 512], FP8H, tag="xh",
                                  name=f"xh_sb_{s}_{ci}")
                xl_sb = xg_p.tile([128, KP, 2, 512], FP8L, tag="xl",
                                  name=f"xl_sb_{s}_{ci}")
                nc.sync.dma_start(xh_sb[:], xqh_r[:, :, :, t0:t0 + 512])
                nc.sync.dma_start(xl_sb[:], xql_r[:, :, :, t0:t0 + 512])

                hh_sb = h8_p.tile([128, FC, 512], FP8H, tag="hh",
                                  name=f"hh_sb_{s}_{ci}")
                hl_sb = h8_p.tile([128, FC, 512], FP8L, tag="hl",
                                  name=f"hl_sb_{s}_{ci}")

                terms = ((wh13, xh_sb), (wh13, xl_sb), (wl13, xh_sb))
                for pa in range(2):  # two fi passes -> 4 PSUM banks each
                    fis = (0, 1) if pa == 0 else (2, 3)
                    psg = {fi: ps_pool.tile([128, 512], F32, tag="ps",
                                            name=f"psg_{s}_{ci}_{fi}")
                           for fi in fis}
                    psu = {fi: ps_pool.tile([128, 512], F32, tag="ps",
                                            name=f"psu_{s}_{ci}_{fi}")
                           for fi in fis}

                    def s1(fi, kp, ti, wsb, xsb):
                        first = (kp == 0 and ti == 0)
                        last = (kp == KP - 1 and ti == 2)
                        gc = (fi // 2) * 512 + (fi % 2) * 128
                        uc = gc + 256
                        nc.tensor.matmul(
                            psg[fi][:, :tl],
                            lhsT=wsb[:, kp, :, gc:gc + 128],
                            rhs=xsb[:, kp, :, :tl],
                            start=first, stop=last, perf_mode=DR)
                        nc.tensor.matmul(
                            psu[fi][:, :tl],
                            lhsT=wsb[:, kp, :, uc:uc + 128],
                            rhs=xsb[:, kp, :, :tl],
                            start=first, stop=last, perf_mode=DR)

                    if s == 0 and ci == 0:
                        # term-major: WhXh first (loads land earliest),
                        # then WlXh, then WhXl
                        t0_terms = ((wh13, xh_sb), (wl13, xh_sb),
                                    (wh13, xl_sb))
                        for ti, (wsb, xsb) in enumerate(t0_terms):
                            for kp in range(KP):
                                for fi in fis:
                                    s1(fi, kp, ti, wsb, xsb)
                    else:
                        for fi in fis:
                            for kp in range(KP):
                                for ti, (wsb, xsb) in enumerate(terms):
                                    s1(fi, kp, ti, wsb, xsb)

                    for fi in fis:
                        sg = sg_p.tile([128, 512], F32, tag="sg",
                                       name=f"sg_{s}_{ci}_{fi}")
                        nc.scalar.activation(sg[:, :tl], psg[fi][:, :tl],
                                             SILU, scale=1.0 / SW1)
                        h32 = h32_p.tile([128, 512], F32, tag="h32",
                                         name=f"h32_{s}_{ci}_{fi}")
                        nc.vector.tensor_tensor(
                            h32[:, :tl], sg[:, :tl], psu[fi][:, :tl],
                            mybir.AluOpType.mult)
                        # identity on-device (|h|<=110); keeps the fp8
                        # cast finite under cost-model simulation where
                        # DoubleRow values are approximate
                        nc.vector.tensor_scalar(
                            h32[:, :tl], h32[:, :tl], 224.0, -224.0,
                            op0=mybir.AluOpType.min,
                            op1=mybir.AluOpType.max)
                        nc.vector.tensor_copy(hh_sb[:, fi, :tl],
                                               h32[:, :tl])
                        nc.vector.tensor_tensor(
                            hl_sb[:, fi, :tl], h32[:, :tl],
                            hh_sb[:, fi, :tl], mybir.AluOpType.subtract)
                return hh_sb, hl_sb

            def stage2(s, ci, t0, tl, hh_sb, hl_sb, final=False):
                # transposed: yT[h, t] = sum_f w2T[f, h] h[f, t]
                _, _, wh2, wl2 = slot_w[s]
                y_lo = y_p.tile([128, HC // 2, 512], BF16, tag="ylo",
                                name=f"y_lo_{s}_{ci}")
                y_hi = y_p.tile([128, HC // 2, 512], BF16, tag="yhi",
                                name=f"y_hi_{s}_{ci}")
                t2 = ((wh2, hh_sb), (wh2, hl_sb), (wl2, hh_sb))
                for hc in range(HC):
                    psy = ps_pool.tile([128, 512], F32, tag="ps",
                                       name=f"psy_{s}_{ci}_{hc}")
                    i = 0
                    for j in range(FP2):
                        for (w2sb, hsb) in t2:
                            nc.tensor.matmul(
                                psy[:, :tl],
                                lhsT=w2sb[:, j, :,
                                          hc * 128:(hc + 1) * 128],
                                rhs=hsb[:, 2 * j:2 * j + 2, :tl],
                                start=(i == 0), stop=(i == 3 * FP2 - 1),
                                perf_mode=DR)
                            i += 1
                    y_sb = y_lo if hc < HC // 2 else y_hi
                    yc = hc % (HC // 2)
                    if hc % 2 == 0:
                        nc.scalar.activation(y_sb[:, yc, :tl],
                                             psy[:, :tl], COPY)
                    else:
                        nc.vector.tensor_copy(y_sb[:, yc, :tl],
                                              psy[:, :tl])
                nc.gpsimd.dma_start(ygt_r[:, :HC // 2, t0:t0 + tl],
                                    y_lo[:, :, :tl])
                if final:
                    # keep the very last DMA tiny: its post-transfer latency
                    # (init + completion-sem) gates the kernel end
                    nc.sync.dma_start(ygt_r[:, HC // 2:HC - 1, t0:t0 + tl],
                                      y_hi[:, :HC // 2 - 1, :tl])
                    nc.gpsimd.dma_start(ygt_r[:, HC - 1:, t0:t0 + tl],
                                        y_hi[:, HC // 2 - 1:, :tl])
                else:
                    nc.sync.dma_start(ygt_r[:, HC // 2:, t0:t0 + tl],
                                      y_hi[:, :, :tl])

            # software pipeline: stage2 lags one chunk behind stage1 so the
            # PE always has stage-1 work while h planes drain
            load_slot_weights(0)
            prev = None
            for (s, ci, t0, tl) in chunk_list:
                h_pair = stage1(s, ci, t0, tl)
                if ci == 0 and s + 1 < S:
                    load_slot_weights(s + 1)
                if prev is not None:
                    stage2(*prev)
                prev = (s, ci, t0, tl) + h_pair
            stage2(*prev, final=True)

    nc.compile()
    return nc


def _ffn_nc(caps):
    key = ("ffn", caps)
    if key not in _nc_cache:
        _nc_cache[key] = _build_ffn(caps)
    return _nc_cache[key]


# ---------------------------------------------------------------------------
# Host orchestration
# ---------------------------------------------------------------------------

def kernel(hidden_states, gate_w, bias, w1, w3, w2):
    x = np.ascontiguousarray(np.asarray(hidden_states, dtype=np.float32))
    gate_w = np.asarray(gate_w, dtype=np.float32)
    bias = np.asarray(bias, dtype=np.float32)
    w1 = np.asarray(w1, dtype=np.float32)
    w3 = np.asarray(w3, dtype=np.float32)
    w2 = np.asarray(w2, dtype=np.float32)

    # ---- Routing on host (fp32) ----
    logits = x @ gate_w.T                               # [T, E]
    scores = 1.0 / (1.0 + np.exp(-logits))
    biased = scores + bias[None, :]
    topi = np.argpartition(-biased, TOPK - 1, axis=1)[:, :TOPK]  # [T, K]
    topw = np.take_along_axis(scores, topi, axis=1)
    topw = topw / topw.sum(axis=1, keepdims=True)
    combine = np.zeros((T, E), dtype=np.float32)
    np.put_along_axis(combine, topi, topw, axis=1)
    idx_per_e = [np.nonzero(combine[:, e] > 0.0)[0] for e in range(E)]
    counts = np.array([len(ix) for ix in idx_per_e])

    # ---- Slot planning ----
    caps, assign = _plan_slots(counts)
    global LAST_CAPS
    LAST_CAPS = caps
    S = len(caps)
    CT = sum(caps)

    # ---- hi/lo fp8 planes ----
    xT = np.ascontiguousarray(x.T)                      # [H, T]
    xTh = xT.astype(E4)
    xTl = (xT - xTh.astype(np.float32)).astype(E5)

    experts_used = sorted({bn[0] for core in assign for bn in core if bn})
    wplanes = {}
    for e in experts_used:
        # half-major layout: [g(0:256) u(0:256) g(256:512) u(256:512)]
        w1t = w1[e].T * SW1
        w3t = w3[e].T * SW3
        w13 = np.concatenate([w1t[:, :256], w3t[:, :256],
                              w1t[:, 256:], w3t[:, 256:]],
                             axis=1)                    # [H, 2F]
        w13h = w13.astype(E4)
        w13l = (w13 - w13h.astype(np.float32)).astype(E5)
        w2t = np.ascontiguousarray(w2[e].T) * SW2       # [F, H]
        w2h = w2t.astype(E4)
        w2l = (w2t - w2h.astype(np.float32)).astype(E5)
        wplanes[e] = (np.ascontiguousarray(w13h), np.ascontiguousarray(w13l),
                      np.ascontiguousarray(w2h), np.ascontiguousarray(w2l))
    e0 = experts_used[0] if experts_used else 0
    if e0 not in wplanes:
        z13 = np.zeros((H, 2 * F), dtype=E4)
        wplanes[e0] = (z13, np.zeros((H, 2 * F), dtype=E5),
                       np.zeros((F, H), dtype=E4), np.zeros((F, H), dtype=E5))

    # ---- Per-core inputs ----
    in_maps = []
    for c in range(NCORES):
        idx_pad = np.zeros(CT, dtype=np.int64)
        for s in range(S):
            bn = assign[c][s]
            if bn is None:
                continue
            e, toff, n = bn
            soff = sum(caps[:s])
            idx_pad[soff:soff + n] = idx_per_e[e][toff:toff + n]
        idx_full = np.zeros(CT + 512, dtype=np.int64)
        idx_full[:CT] = idx_pad
        im = {
            "xqh": np.ascontiguousarray(xTh[:, idx_full]),
            "xql": np.ascontiguousarray(xTl[:, idx_full]),
        }
        for s in range(S):
            bn = assign[c][s]
            e = bn[0] if bn is not None else e0
            p13h, p13l, p2h, p2l = wplanes[e]
            im[f"w13h{s}"] = p13h
            im[f"w13l{s}"] = p13l
            im[f"w2h{s}"] = p2h
            im[f"w2l{s}"] = p2l
        in_maps.append(im)

    # ---- Device FFN ----
    ncf = _ffn_nc(caps)
    res = run_bass_kernel_spmd(ncf, in_maps, core_ids=list(range(NCORES)))

    # ---- Host combine + scatter-add ----
    out = np.zeros((T, H), dtype=np.float32)
    for c in range(NCORES):
        ygt = res.results[c]["ygt"]                     # [H, CT] bf16
        for s in range(S):
            bn = assign[c][s]
            if bn is None:
                continue
            e, toff, n = bn
            soff = sum(caps[:s])
            ix = idx_per_e[e][toff:toff + n]
            yrows = ygt[:, soff:soff + n].astype(np.float32).T  # [n, H]
            out[ix] += (combine[ix, e] * DESCALE)[:, None] * yrows
    return out
